# revision 1
# baseline (speedup 1.0000x reference)
"""Cross-attention (RMSNorm + QKV proj + 2D RoPE + SDPA + out-proj) on 8
Trainium2 NeuronCores.

Sharding: 8 cores = 4 batches x 2 query-halves. Each core computes the full
KV projection for its batch (duplicated across the 2 cores sharing a batch)
and attention + output projection for its 512 query rows. No collectives.

On-device layout is feature-major ("transposed"): activations live as
[feature, seq] with features on SBUF partitions. Host pre-transposes inputs
and weights so every linear layer is a plain lhsT.T @ rhs PE matmul whose
output is again feature-major. Head dims are de-interleaved (even rot dims
then odd rot dims per head) so RoPE's pair rotation becomes a 32-partition
block swap (SBUF->SBUF DMA) plus elementwise DVE math, with the sin sign
folded into host-negated frequency rows.

Matmuls run in float32r mode (full PE rate, ~1.5e-4 rel err); the RoPE angle
matmuls stay fp32 exact. Softmax skips max-subtraction (logits are O(1) by
construction) and the denominator comes free as a 65th ones-column in the
AV matmul; normalization happens before the output projection.
"""

import numpy as np

B, SQ, SK, D = 4, 1024, 1024, 768
H, HD = 12, 64
DC = D // 128          # 6 feature chunks
SQL = SQ // 2          # 512 query rows per core
SKC = SK // 128        # 8 key chunks
EPS = 1e-5
PI = float(np.pi)
TWOPI = 2.0 * PI
INV2PI = 1.0 / TWOPI
RBIG = 12582912.0      # 1.5 * 2**23: fp32 round-to-nearest-integer trick
NCORES = 8

_cache = {}


# ---------------------------------------------------------------------------
# compiler workarounds
# ---------------------------------------------------------------------------

def _apply_patches():
    """This walrus build allows only ONE sync-wait command per instruction.
    (a) split the Tile kernel-tail drain into one drain per waited proc;
    (b) post-process the BIR JSON, moving excess waits onto same-engine NoOps
    inserted immediately before the over-subscribed instruction."""
    import json
    import concourse.tile as tile
    import concourse.bass as cbass
    from concourse.vector_clock import ScopedClock, VectorClock

    if getattr(cbass.Bass, "_wait_split_patched", False):
        return

    def _drain_and_barrier(self, tick_clock, wait_clock):
        gc = tick_clock.global_clock
        try:
            vec = gc[None]
        except Exception:
            vec = gc
        n = len(vec)
        for p in [i for i in range(n) if vec[i] > 0]:
            sub = [0] * n
            sub[p] = vec[p]
            inst = self.nc.sync.drain()
            wait_clock.add_sem_waits(inst.ins, ScopedClock({None: VectorClock(sub)}))
        self.nc.all_engine_barrier()
        assert self.sems is not None
        popped = self.nc._tile_sem_poison_stack.pop()
        assert popped is self._sem_poison
        self.nc.clear_and_free_semaphores(list(self.sems.allocated().values()))
        self.nc.all_engine_barrier()

    tile.TileContext._drain_and_barrier = _drain_and_barrier

    def _split_waits(bir):
        for f in bir.get("functions", []):
            for blk in f.get("blocks", []):
                insts = blk.get("instructions")
                if not insts:
                    continue
                out = []
                ctr = 0
                for inst in insts:
                    si = inst.get("sync_info")
                    ow = (si or {}).get("on_wait") or []
                    if len(ow) > 1:
                        for w in ow[:-1]:
                            nop = {
                                "name": f"{inst['name']}-ws{ctr}",
                                "opcode": "NoOp",
                                "engine": inst.get("engine"),
                                "ins": [],
                                "outs": [],
                                "sync_info": {"on_wait": [w], "on_update": []},
                            }
                            if "debug" in inst:
                                nop["debug"] = inst["debug"]
                            ctr += 1
                            out.append(nop)
                        si["on_wait"] = [ow[-1]]
                    out.append(inst)
                blk["instructions"] = out
        return bir

    orig = cbass.Bass.to_json_bytes

    def to_json_bytes(self, *a, **kw):
        return json.dumps(_split_waits(json.loads(orig(self, *a, **kw)))).encode()

    cbass.Bass.to_json_bytes = to_json_bytes
    cbass.Bass._wait_split_patched = True


# ---------------------------------------------------------------------------
# device program
# ---------------------------------------------------------------------------

def _build_nc():
    import concourse.bass as bass
    import concourse.tile as tile
    import concourse.mybir as mybir

    F32 = mybir.dt.float32
    F32R = mybir.dt.float32r
    AF = mybir.ActivationFunctionType
    ALU = mybir.AluOpType

    nc = bass.Bass()

    qT_d = nc.dram_tensor("qT", [D, SQL], F32R, kind="ExternalInput")
    kvT_d = nc.dram_tensor("kvT", [D, SK], F32R, kind="ExternalInput")
    posq_d = nc.dram_tensor("posqT", [2, SQL], F32, kind="ExternalInput")
    posk_d = nc.dram_tensor("poskT", [2, SK], F32, kind="ExternalInput")
    freqs_d = nc.dram_tensor("freqsT", [2, D], F32, kind="ExternalInput")
    wq_d = nc.dram_tensor("wqT", [D, D], F32R, kind="ExternalInput")
    wk_d = nc.dram_tensor("wkT", [D, D], F32R, kind="ExternalInput")
    wv_d = nc.dram_tensor("wvT", [D, D], F32R, kind="ExternalInput")
    wo_d = nc.dram_tensor("woT", [D, D], F32R, kind="ExternalInput")
    bq_d = nc.dram_tensor("bqR", [128, DC], F32, kind="ExternalInput")
    bk_d = nc.dram_tensor("bkR", [128, DC], F32, kind="ExternalInput")
    bo_d = nc.dram_tensor("boR", [128, DC], F32, kind="ExternalInput")
    ones_d = nc.dram_tensor("ones128", [128, 128], F32R, kind="ExternalInput")
    onesc_d = nc.dram_tensor("onescol", [128, H], F32R, kind="ExternalInput")
    out_d = nc.dram_tensor("outT", [D, SQL], F32, kind="ExternalOutput")

    den_d = nc.dram_tensor("den_scratch", [H, SQL], F32, kind="Internal")

    with tile.TileContext(nc) as tc:
        import contextlib
        ctx = contextlib.ExitStack()
        with ctx:
            persist = ctx.enter_context(tc.tile_pool(name="persist", bufs=1))
            tmp = ctx.enter_context(tc.tile_pool(name="tmp", bufs=2))
            ps = ctx.enter_context(tc.tile_pool(name="ps", bufs=6, space="PSUM"))
            pso = ctx.enter_context(tc.tile_pool(name="pso", bufs=2, space="PSUM"))

            # ---- persistent small tensors -------------------------------
            freqs_sb = persist.tile([2, D], F32)
            posq_sb = persist.tile([2, SQL], F32)
            posk_sb = persist.tile([2, SK], F32)
            bq_sb = persist.tile([128, DC], F32)
            bk_sb = persist.tile([128, DC], F32)
            bo_sb = persist.tile([128, DC], F32)
            ones_sb = persist.tile([128, 128], F32R)
            onesc_sb = persist.tile([128, H], F32R)
            halfpi = persist.tile([128, 1], F32)
            eps_t = persist.tile([128, 1], F32)
            den12 = persist.tile([H, SQL], F32)
            nc.sync.dma_start(out=freqs_sb, in_=freqs_d[:, :])
            nc.sync.dma_start(out=posq_sb, in_=posq_d[:, :])
            nc.sync.dma_start(out=posk_sb, in_=posk_d[:, :])
            nc.sync.dma_start(out=bq_sb, in_=bq_d[:, :])
            nc.sync.dma_start(out=bk_sb, in_=bk_d[:, :])
            nc.sync.dma_start(out=bo_sb, in_=bo_d[:, :])
            nc.sync.dma_start(out=ones_sb, in_=ones_d[:, :])
            nc.sync.dma_start(out=onesc_sb, in_=onesc_d[:, :])
            nc.vector.memset(halfpi, PI / 2)
            nc.vector.memset(eps_t, EPS)

            # ---- persistent activations ---------------------------------
            qrot = [persist.tile([128, SQL], F32R, name=f"qrot{c}") for c in range(DC)]
            krot = [persist.tile([128, SK], F32R, name=f"krot{c}") for c in range(DC)]
            vp = [persist.tile([128, H, HD + 1], F32R, name=f"vp{c}") for c in range(SKC)]
            oT = [persist.tile([128, SQL], F32R, name=f"oT{c}") for c in range(DC)]

            # ---- phase-1 inputs -----------------------------------------
            ph1 = ctx.enter_context(tc.tile_pool(name="ph1", bufs=1))
            qT = [ph1.tile([128, SQL], F32R, name=f"qT{c}") for c in range(DC)]
            kvT = [ph1.tile([128, SK], F32R, name=f"kvT{c}") for c in range(DC)]
            for c in range(DC):
                nc.sync.dma_start(out=qT[c], in_=qT_d[c * 128:(c + 1) * 128, :])
                nc.sync.dma_start(out=kvT[c], in_=kvT_d[c * 128:(c + 1) * 128, :])

            def wchunk(dram, c):
                t = tmp.tile([128, D], F32R, tag="wchunk", name="wch")
                nc.sync.dma_start(out=t, in_=dram[c * 128:(c + 1) * 128, :])
                return t

            # ---- RMSNorm over features (partition dim) ------------------
            ss = ps.tile([128, SQL], F32, tag="p512")
            for c in range(DC):
                sq = tmp.tile([128, SQL], F32R, tag="e512", bufs=4, name="sq")
                nc.vector.tensor_mul(out=sq, in0=qT[c], in1=qT[c])
                nc.tensor.matmul(ss, ones_sb, sq, start=(c == 0), stop=(c == DC - 1))
            sq_t = tmp.tile([128, SQL], F32, tag="outc")
            nc.scalar.activation(out=sq_t, in_=ss, func=AF.Sqrt, bias=eps_t,
                                 scale=1.0 / D)
            rstd = tmp.tile([128, SQL], F32, tag="outc", name="rstd")
            nc.vector.reciprocal(out=rstd, in_=sq_t)
            for c in range(DC):
                nc.vector.tensor_mul(out=qT[c], in0=qT[c], in1=rstd)

            # ---- helpers ------------------------------------------------
            def rope_reduce(ps_ang, width, sin_dst, cos_dst):
                """sin/cos of angles in ps_ang [128,width], with fp32
                round-to-nearest range reduction into [-pi, pi]."""
                t2 = tmp.tile([128, 512], F32, tag="rr", bufs=3, name="t2")[:, :width]
                nc.vector.tensor_scalar(out=t2, in0=ps_ang, scalar1=INV2PI,
                                        scalar2=RBIG, op0=ALU.mult, op1=ALU.add)
                kk = tmp.tile([128, 512], F32, tag="rr", bufs=3, name="kk")[:, :width]
                nc.vector.tensor_scalar_add(out=kk, in0=t2, scalar1=-RBIG)
                red = tmp.tile([128, 512], F32, tag="rr", bufs=3, name="red")[:, :width]
                nc.vector.scalar_tensor_tensor(out=red, in0=kk, scalar=-TWOPI,
                                               in1=ps_ang, op0=ALU.mult,
                                               op1=ALU.add)
                nc.scalar.activation(out=sin_dst, in_=red, func=AF.Sin)
                t2c = tmp.tile([128, 512], F32, tag="rr", bufs=3, name="t2c")[:, :width]
                nc.vector.tensor_scalar(out=t2c, in0=ps_ang, scalar1=INV2PI,
                                        scalar2=RBIG + 0.25, op0=ALU.mult,
                                        op1=ALU.add)
                kkc = tmp.tile([128, 512], F32, tag="rr", bufs=3, name="kkc")[:, :width]
                nc.vector.tensor_scalar_add(out=kkc, in0=t2c, scalar1=-RBIG)
                redc = tmp.tile([128, 512], F32, tag="rr", bufs=3, name="redc")[:, :width]
                nc.vector.scalar_tensor_tensor(out=redc, in0=kkc, scalar=-TWOPI,
                                               in1=ps_ang, op0=ALU.mult,
                                               op1=ALU.add)
                nc.scalar.activation(out=cos_dst, in_=redc, func=AF.Sin,
                                     bias=halfpi)

            def block_swap(dst, src, width):
                for base in (0, 64):
                    nc.gpsimd.dma_start(out=dst[base:base + 32, :width],
                                        in_=src[base + 32:base + 64, :width])
                    nc.gpsimd.dma_start(out=dst[base + 32:base + 64, :width],
                                        in_=src[base:base + 32, :width])

            # ---- Q projection (c-outer, 6 psum accumulators) ------------
            pq6 = [ps.tile([128, 512], F32, tag="p512", name=f"pq{m}")
                   for m in range(DC)]
            for c in range(DC):
                wc = wchunk(wq_d, c)
                for m in range(DC):
                    nc.tensor.matmul(pq6[m], wc[:, m * 128:(m + 1) * 128], qT[c],
                                     start=(c == 0), stop=(c == DC - 1))
            # ---- RoPE on Q (bias copy interleaved per chunk) ------------
            for m in range(DC):
                qp = tmp.tile([128, 512], F32, tag="pre", bufs=3, name="qp")
                nc.scalar.activation(out=qp, in_=pq6[m], func=AF.Identity,
                                     bias=bq_sb[:, m:m + 1])
                pa = ps.tile([128, 512], F32, tag="p512", name="pa")
                nc.tensor.matmul(pa, freqs_sb[:, m * 128:(m + 1) * 128], posq_sb,
                                 start=True, stop=True)
                sin_t = tmp.tile([128, 512], F32, tag="sin", name="sin_t")
                cos_t = tmp.tile([128, 512], F32, tag="cos", name="cos_t")
                rope_reduce(pa, SQL, sin_t, cos_t)
                t1 = tmp.tile([128, 512], F32, tag="cmb", name="t1")
                block_swap(t1, qp, SQL)
                nc.vector.tensor_mul(out=t1, in0=t1, in1=sin_t)
                nc.vector.tensor_mul(out=qrot[m], in0=qp, in1=cos_t)
                nc.vector.tensor_add(out=qrot[m], in0=qrot[m], in1=t1)

            # ---- K projection + RoPE (per 512-half, c-outer) ------------
            for half in range(2):
                hs = slice(half * 512, half * 512 + 512)
                pk6 = [ps.tile([128, 512], F32, tag="p512", name=f"pk{m}")
                       for m in range(DC)]
                for c in range(DC):
                    wc = wchunk(wk_d, c)
                    for m in range(DC):
                        nc.tensor.matmul(pk6[m], wc[:, m * 128:(m + 1) * 128],
                                         kvT[c][:, hs],
                                         start=(c == 0), stop=(c == DC - 1))
                for m in range(DC):
                    kp = tmp.tile([128, 512], F32, tag="pre", bufs=3, name="kp")
                    nc.scalar.activation(out=kp, in_=pk6[m],
                                         func=AF.Identity, bias=bk_sb[:, m:m + 1])
                    pa = ps.tile([128, 512], F32, tag="p512", name="pa")
                    nc.tensor.matmul(pa, freqs_sb[:, m * 128:(m + 1) * 128],
                                     posk_sb[:, hs], start=True, stop=True)
                    sin_t = tmp.tile([128, 512], F32, tag="sin", name="sin_t")
                    cos_t = tmp.tile([128, 512], F32, tag="cos", name="cos_t")
                    rope_reduce(pa, 512, sin_t, cos_t)
                    t1 = tmp.tile([128, 512], F32, tag="cmb", name="t1")
                    block_swap(t1, kp, 512)
                    nc.vector.tensor_mul(out=t1, in0=t1, in1=sin_t)
                    nc.vector.tensor_mul(out=krot[m][:, hs], in0=kp, in1=cos_t)
                    nc.vector.tensor_add(out=krot[m][:, hs], in0=krot[m][:, hs],
                                         in1=t1)

            # ---- V projection (row-major, ones column appended) ---------
            wv = []
            for c in range(DC):
                t = ph1.tile([128, D], F32R, tag=f"wv{c}")
                nc.sync.dma_start(out=t, in_=wv_d[c * 128:(c + 1) * 128, :])
                wv.append(t)
            for kc in range(SKC):
                ksl = slice(kc * 128, (kc + 1) * 128)
                pv0 = ps.tile([128, 512], F32, tag="p512")
                pv1 = ps.tile([128, 512], F32, tag="p512")
                for c in range(DC):
                    nc.tensor.matmul(pv0, kvT[c][:, ksl], wv[c][:, 0:512],
                                     start=(c == 0), stop=(c == DC - 1))
                    nc.tensor.matmul(pv1[:, 0:256], kvT[c][:, ksl],
                                     wv[c][:, 512:768],
                                     start=(c == 0), stop=(c == DC - 1))
                nc.vector.tensor_copy(
                    out=vp[kc][:, 0:8, 0:HD],
                    in_=pv0.rearrange("p (h d) -> p h d", h=8))
                nc.vector.tensor_copy(
                    out=vp[kc][:, 8:12, 0:HD],
                    in_=pv1[:, 0:256].rearrange("p (h d) -> p h d", h=4))
                nc.gpsimd.dma_start(out=vp[kc][:, :, HD], in_=onesc_sb)

            # ---- attention ----------------------------------------------
            for h in range(H):
                mh, off = h // 2, 64 * (h % 2)
                po = pso.tile([65, 512], F32, tag="po", name="po")
                for kc in range(SKC):
                    pss = ps.tile([128, 512], F32, tag="p512", name="pss")
                    nc.tensor.matmul(
                        pss,
                        krot[mh][off:off + 64, kc * 128:(kc + 1) * 128],
                        qrot[mh][off:off + 64, :],
                        start=True, stop=True)
                    ex = tmp.tile([128, SQL], F32R, tag="e512", bufs=4, name="ex")
                    nc.scalar.activation(out=ex, in_=pss, func=AF.Exp,
                                         scale=1.0 / 8.0)
                    nc.tensor.matmul(po, vp[kc][:, h, :], ex,
                                     start=(kc == 0), stop=(kc == SKC - 1))
                # stash unnormalized O and the denominator row
                nc.vector.tensor_copy(out=oT[mh][off:off + 64, :], in_=po[0:64, :])
                drow = tmp.tile([1, SQL], F32, tag="drow", name="drow")
                nc.vector.tensor_copy(out=drow, in_=po[64:65, :])
                nc.sync.dma_start(out=den12[h:h + 1, :], in_=drow)

            # normalize: batched reciprocal + partition-broadcast via DRAM
            nc.vector.reciprocal(out=den12, in_=den12)
            nc.sync.dma_start(out=den_d[:, :], in_=den12)
            import concourse.bass as bass_mod
            for mh in range(DC):
                rb2 = tmp.tile([128, SQL], F32, tag="outc", name="rb2")
                for j in range(2):
                    row = den_d[2 * mh + j, :]
                    bsrc = bass_mod.AP(tensor=row.tensor, offset=row.offset,
                                       ap=[[0, 64], *row.ap])
                    nc.sync.dma_start(out=rb2[64 * j:64 * j + 64, :], in_=bsrc)
                nc.vector.tensor_mul(out=oT[mh], in0=oT[mh], in1=rb2)

            # ---- output projection (c-outer, 6 psum accumulators) -------
            po6 = [ps.tile([128, 512], F32, tag="p512", name=f"po6_{m}")
                   for m in range(DC)]
            for c in range(DC):
                wc = wchunk(wo_d, c)
                for m in range(DC):
                    nc.tensor.matmul(po6[m], wc[:, m * 128:(m + 1) * 128], oT[c],
                                     start=(c == 0), stop=(c == DC - 1))
            for m in range(DC):
                outc = tmp.tile([128, SQL], F32, tag="outc", name="outc")
                nc.scalar.activation(out=outc, in_=po6[m], func=AF.Identity,
                                     bias=bo_sb[:, m:m + 1])
                nc.sync.dma_start(out=out_d[m * 128:(m + 1) * 128, :], in_=outc)

    return nc


# ---------------------------------------------------------------------------
# host wrapper
# ---------------------------------------------------------------------------

def kernel(q, kv, posq, posk, w_norm, w_q, b_q, w_kv, b_kv, w_out, b_out, freqs):
    _apply_patches()
    from concourse.bass_utils import run_bass_kernel_spmd

    q = np.asarray(q, np.float32)
    kv = np.asarray(kv, np.float32)
    posq_np = np.asarray(posq)
    posk_np = np.asarray(posk)
    w_norm = np.asarray(w_norm, np.float32)
    w_q = np.asarray(w_q, np.float32)
    b_q = np.asarray(b_q, np.float32)
    w_kv = np.asarray(w_kv, np.float32)
    b_kv = np.asarray(b_kv, np.float32)
    w_out = np.asarray(w_out, np.float32)
    b_out = np.asarray(b_out, np.float32)
    freqs = np.asarray(freqs, np.float32)

    # de-interleave head dims: new j<32 -> old 2j (even), j>=32 -> old 2(j-32)+1
    perm = np.empty(D, np.int64)
    for h in range(H):
        for j in range(HD):
            perm[h * HD + j] = h * HD + (2 * j if j < 32 else 2 * (j - 32) + 1)

    wqT = np.ascontiguousarray((w_q[perm, :] * w_norm[None, :]).T)
    wkT = np.ascontiguousarray(w_kv[:D][perm, :].T)
    wvT = np.ascontiguousarray(w_kv[D:].T)
    woT = np.ascontiguousarray(w_out.T)
    bqR = np.ascontiguousarray(b_q[perm].reshape(DC, 128).T)
    bkR = np.ascontiguousarray(b_kv[:D][perm].reshape(DC, 128).T)
    bo_eff = b_out + w_out @ b_kv[D:]          # fold V bias (softmax sums to 1)
    boR = np.ascontiguousarray(bo_eff.reshape(DC, 128).T)

    # frequency rows in de-interleaved layout; e-rows negated so that
    # sin(ang_signed) carries the rotation sign
    fr = np.empty((2, D), np.float32)
    for h in range(H):
        f = freqs[:, h, :]                      # [2, 32]
        fr[:, h * HD:h * HD + 32] = -f
        fr[:, h * HD + 32:(h + 1) * HD] = f

    ones128 = np.ones((128, 128), np.float32)
    onescol = np.ones((128, H), np.float32)

    if "nc" not in _cache:
        _cache["nc"] = _build_nc()
    nc = _cache["nc"]

    in_maps = []
    for core in range(NCORES):
        b, half = core // 2, core % 2
        qs = slice(half * SQL, (half + 1) * SQL)
        in_maps.append({
            "qT": np.ascontiguousarray(q[b, qs, :].T),
            "kvT": np.ascontiguousarray(kv[b].T),
            "posqT": np.ascontiguousarray(posq_np[b, qs, :].T.astype(np.float32)),
            "poskT": np.ascontiguousarray(posk_np[b].T.astype(np.float32)),
            "freqsT": fr,
            "wqT": wqT, "wkT": wkT, "wvT": wvT, "woT": woT,
            "bqR": bqR, "bkR": bkR, "boR": boR,
            "ones128": ones128, "onescol": onescol,
        })

    res = run_bass_kernel_spmd(nc, in_maps, core_ids=list(range(NCORES)))
    kernel._last_result = res

    out = np.empty((B, SQ, D), np.float32)
    for core in range(NCORES):
        b, half = core // 2, core % 2
        out[b, half * SQL:(half + 1) * SQL, :] = res.results[core]["outT"].T
    return out

